# revision 44
# baseline (speedup 1.0000x reference)
"""Multi-head attention (B=2, N=2048, D=1024, 16 heads x 64) on 8 NeuronCores.

Sharding: data-parallel over batch (2) x tensor-parallel over heads (4 heads
per core). Each core computes q/k/v projections + RoPE + attention for its 4
heads and a partial output projection; the host sums the 4 tensor-parallel
partials per batch and adds the output bias (V-projection bias folded in).

Kernel structure (cost-model-driven):
 - Projections in f32r (full-rate at free>=256); RoPE rotate-pair via a
   channel-permuted eye matmul (permutation folded into weights host-side).
 - Scores computed transposed S^T[k, q] from bf16 q/k; exp on ACT with a
   constant -20 bias (cancels in the softmax ratio); es output in bf16.
 - ctx matmuls flipped to out[q, d] orientation (out partitions = q tokens,
   free = 65 = 64 v-cols + ones column for the denominator) in bf16: bf16
   streams 1 row/cycle at any free size, halving ctx PE time vs the
   [65, q]-oriented f32r version.
 - Softmax denominators land per-partition -> normalize is a cheap DVE
   reciprocal + tensor_scalar multiply; normalized ctx transposed for the
   output projection by single-shot matmuls against a bf16 identity (the
   DMA xbar transpose's output is invisible to the dependency scheduler).
 - Output projection in bf16; bf16 partials DMA'd out, host sums in f32.
 - The ACT exp wall (~133us) is the global bottleneck: scores+exp for all
   of q-chunk 0 are emitted during the projection phase (es tiles piled in
   SBUF, 256-token projection chunks free the SBUF for the pile), and wave
   B runs a software-pipelined head chain: head i's single-shot score
   matmuls interleave with head i-1's contiguous ctx accumulation groups
   (hw allows only one open psum accumulation group at a time), with
   o-proj pieces dripped into the remaining slots. The final head fuses
   ctx/normalize/transpose/o-proj per q-tile to shorten the tail.
"""
import sys

sys.path.insert(0, "/opt/trn_rl_repo")

import numpy as np

import concourse.bacc as bacc
import concourse.mybir as mybir
import concourse.tile as tile
from concourse import bass_utils

B, N, D = 2, 2048, 1024
HEADS, HD = 16, 64
TP = 4                 # tensor-parallel ways (heads)
DP = 2                 # data-parallel ways (batch)
HPC = HEADS // TP      # heads per core = 4
C = HPC * HD           # channels per core = 256
CH = 512               # q-chunk size (attention)
NCH = N // CH          # 4
PCH = 256              # projection x-chunk size (phase A)
NPCH = N // PCH        # 8
KT = 128               # k tile
NKT = N // KT          # 16
GK = 2                 # k-tiles per exp group
NGR = NKT // GK        # 8
VW = HD + 1            # V columns per head incl. ones column = 65
ITC = D // KT          # 8 contraction tiles for projections
F32R = mybir.dt.float32r
F32 = mybir.dt.float32
BF16 = mybir.dt.bfloat16

_CACHE = {}


def _build():
    nc = bacc.Bacc("TRN2", debug=False, num_devices=DP * TP)

    xT = nc.dram_tensor("xT", [D, N], F32R, kind="ExternalInput").ap()
    cosT = nc.dram_tensor("cosT", [C, N], F32R, kind="ExternalInput").ap()
    sinT = nc.dram_tensor("sinT", [C, N], F32R, kind="ExternalInput").ap()
    wq = nc.dram_tensor("wq", [D, C], F32R, kind="ExternalInput").ap()
    wk = nc.dram_tensor("wk", [D, C], F32R, kind="ExternalInput").ap()
    wvx = nc.dram_tensor("wvx", [D, HPC * VW], F32R, kind="ExternalInput").ap()
    bqk = nc.dram_tensor("bqk", [2, 2, 128], F32, kind="ExternalInput").ap()
    woT = nc.dram_tensor("woT", [C, D], BF16, kind="ExternalInput").ap()
    eyesw = nc.dram_tensor("eyesw", [128, 128], F32R, kind="ExternalInput").ap()
    eyebf = nc.dram_tensor("eyebf", [128, 128], BF16, kind="ExternalInput").ap()
    out = nc.dram_tensor("out", [N, D], BF16, kind="ExternalOutput").ap()

    with tile.TileContext(nc) as tc:
        with tc.tile_pool(name="pers", bufs=1) as pers, \
             tc.tile_pool(name="wrk", bufs=1) as wrk, \
             tc.tile_pool(name="psp", bufs=1, space="PSUM") as psp:
            # ---- persistent SBUF; DMA order = arrival priority: the rope
            # chain of chunk 0 gates the first exp, so wk/x0/wq/bqk/eye/cs0
            # land first and everything else queues behind ----
            bqk_sb = pers.tile([128, 2, 2], F32, tag="bqk")
            nc.sync.dma_start(bqk_sb[:], bqk.rearrange("a c p -> p a c"))
            eye_sb = pers.tile([128, 128], F32R, tag="eyesw")
            nc.sync.dma_start(eye_sb[:], eyesw)
            # wk and x0 stream in quarters so the warmup matmul batches
            # below never let the PE go idle (an idle->busy transition
            # resets the p-state ramp and the next dispatches run at half
            # clock)
            wk_sb = pers.tile([128, ITC, C], F32R, tag="wk")
            for qtr in range(4):
                nc.sync.dma_start(
                    wk_sb[:, 2 * qtr:2 * qtr + 2, :],
                    wk[256 * qtr:256 * (qtr + 1), :].rearrange(
                        "(t p) c -> p t c", p=128))
            xt0 = wrk.tile([128, ITC, PCH], F32R, tag="xt", bufs=3, name="xt0")
            for qtr in range(4):
                nc.sync.dma_start(
                    xt0[:, 2 * qtr:2 * qtr + 2, :],
                    xT[256 * qtr:256 * (qtr + 1), 0:PCH].rearrange(
                        "(t p) n -> p t n", p=128))

            def load_cs(nch):
                cs = []
                ns = slice(nch * PCH, (nch + 1) * PCH)
                for t in range(2):
                    co = wrk.tile([128, PCH], F32R, tag=f"cos{t}", bufs=2,
                                  name=f"cos{t}_{nch}")
                    nc.sync.dma_start(co[:], cosT[128 * t:128 * (t + 1), ns])
                    si = wrk.tile([128, PCH], F32R, tag=f"sin{t}", bufs=2,
                                  name=f"sin{t}_{nch}")
                    nc.sync.dma_start(si[:], sinT[128 * t:128 * (t + 1), ns])
                    cs.append((co, si))
                return cs

            def load_chunk_x(nch):
                xt = wrk.tile([128, ITC, PCH], F32R, tag="xt", bufs=3)
                ns = slice(nch * PCH, (nch + 1) * PCH)
                nc.sync.dma_start(
                    xt[:, 0:4, :],
                    xT[0:512, ns].rearrange("(t p) n -> p t n", p=128))
                nc.sync.dma_start(
                    xt[:, 4:, :],
                    xT[512:, ns].rearrange("(t p) n -> p t n", p=128))
                return xt

            cs0 = load_cs(0)
            wq_sb = pers.tile([128, ITC, C], F32R, tag="wq")
            nc.sync.dma_start(
                wq_sb[:, 0:4, :],
                wq[0:512, :].rearrange("(t p) c -> p t c", p=128))
            nc.sync.dma_start(
                wq_sb[:, 4:, :],
                wq[512:, :].rearrange("(t p) c -> p t c", p=128))
            xt1 = load_chunk_x(1)
            cs1 = load_cs(1)

            wv_sb = pers.tile([128, ITC, HPC * VW], F32R, tag="wv")
            nc.sync.dma_start(wv_sb[:], wvx.rearrange("(t p) c -> p t c", p=128))
            eyebf_sb = pers.tile([128, 128], BF16, tag="eyebf")
            nc.sync.dma_start(eyebf_sb[:], eyebf)

            shift_sb = pers.tile([128, 1], F32, tag="shift")
            nc.gpsimd.memset(shift_sb[:], -20.0)
            # v tiles: persistent, bf16, ones column memset once
            v_sb = [pers.tile([128, HPC, VW], BF16, tag=f"v{t}", name=f"v{t}")
                    for t in range(NKT)]
            for t in range(NKT):
                nc.gpsimd.memset(v_sb[t][:, :, HD:VW], 1.0)

            # p-state warmup: keep the PE streak alive through the initial
            # DMA wait so the first projections run at full clock; operands
            # track the DMA arrival order (wk first, then the x halves)
            wps = psp.tile([128, CH], F32, tag="aux", bufs=2, name="warmup")
            for qtr in range(4):
                for wi in range(5):
                    nc.tensor.matmul(wps[:, 0:128],
                                     lhsT=wk_sb[:, 2 * qtr, 0:128],
                                     rhs=wk_sb[:, 2 * qtr, 0:128],
                                     start=True, stop=True)
            for qtr in range(4):
                for wi in range(5):
                    nc.tensor.matmul(wps[:, 0:128],
                                     lhsT=wk_sb[:, 0, 0:128],
                                     rhs=xt0[:, 2 * qtr, 0:128],
                                     start=True, stop=True)

            qrot = [pers.tile([128, N], F32R, tag=f"qrot{t}", name=f"qrot{t}")
                    for t in range(2)]
            krot = [pers.tile([128, N], F32R, tag=f"krot{t}", name=f"krot{t}")
                    for t in range(2)]
            ctxT = [pers.tile([128, N], BF16, tag=f"ctxT{t}", name=f"ctxT{t}")
                    for t in range(2)]
            wo_sb = pers.tile([128, 2, D], BF16, tag="wo")

            def proj_rope(w_sb, qk, dst, xt, cs, nch):
                ns = slice(nch * PCH, (nch + 1) * PCH)
                ps, raw = [], []
                for dc in range(2):
                    p = psp.tile([128, CH], F32, tag="aux", bufs=2,
                                 name=f"ps{qk}_{nch}_{dc}")
                    for it in range(ITC):
                        nc.tensor.matmul(
                            p[:, 0:PCH],
                            lhsT=w_sb[:, it, 128 * dc:128 * (dc + 1)],
                            rhs=xt[:, it, :],
                            start=(it == 0), stop=(it == ITC - 1))
                    ps.append(p)
                    r = wrk.tile([128, PCH], F32R, tag="raw", bufs=3,
                                 name=f"raw{qk}_{nch}_{dc}")
                    nc.vector.tensor_scalar_add(
                        r[:], p[:, 0:PCH], bqk_sb[:, qk, dc:dc + 1])
                    raw.append(r)
                for dc in range(2):
                    pssh = psp.tile([128, CH], F32, tag="aux", bufs=2,
                                    name=f"pssh{qk}_{nch}_{dc}")
                    nc.tensor.matmul(pssh[:, 0:PCH], lhsT=eye_sb[:],
                                     rhs=raw[dc][:],
                                     start=True, stop=True)
                    co, si = cs[dc]
                    # m1 and the final add are SBUF-only -> gpsimd, except
                    # for the first two chunks where the rope tail gates the
                    # first scores/exp: run those on the faster DVE. m2
                    # reads PSUM so it always stays on DVE.
                    meng = nc.vector if nch < 2 else nc.gpsimd
                    m1 = wrk.tile([128, PCH], F32, tag="m1", bufs=1)
                    meng.tensor_mul(m1[:], raw[dc][:], co[:])
                    m2 = wrk.tile([128, PCH], F32, tag="m2", bufs=1)
                    nc.vector.tensor_mul(m2[:], pssh[:, 0:PCH], si[:])
                    meng.tensor_add(dst[dc][:, ns], m1[:], m2[:])

            def vproj(xt, nch, vt):
                kt = nch * (PCH // KT) + vt
                psv = psp.tile([128, CH], F32, tag="aux", bufs=2,
                               name=f"psv{kt}")
                for it in range(ITC):
                    nc.tensor.matmul(
                        psv[:, 0:HPC * VW],
                        lhsT=xt[:, it, KT * vt:KT * (vt + 1)],
                        rhs=wv_sb[:, it, :],
                        start=(it == 0), stop=(it == ITC - 1))
                pv = psv[:, 0:HPC * VW].rearrange("p (s w) -> p s w", w=VW)
                nc.vector.tensor_copy(v_sb[kt][:, :, 0:HD], pv[:, :, 0:HD])

            # scores/exp for qc0 heads 0,1,3 run during phase A; their es
            # tiles pile up in SBUF until wave B consumes them. Head 2 leads
            # the wave-B pipeline so that within each head-pair the par=0
            # head normalizes before the par=1 head (the par=1 normalize
            # completes the pair's ctxn tiles and fires the transposes).
            PILE_HEADS = (0, 1, 2, 3)
            es_pile = {h: [None] * NGR for h in PILE_HEADS}

            def scores_group(qc, h, g):
                pt, par = h // 2, h % 2
                r0 = 64 * par
                qs = slice(qc * CH, (qc + 1) * CH)
                stg = psp.tile([128, GK, CH], F32, tag="stg", bufs=2,
                               name=f"stg{qc}_{h}_{g}")
                for j in range(GK):
                    kt = GK * g + j
                    nc.tensor.matmul(
                        stg[:, j, :],
                        lhsT=krot[pt][r0:r0 + 64, KT * kt:KT * (kt + 1)],
                        rhs=qrot[pt][r0:r0 + 64, qs],
                        start=True, stop=True)
                es = wrk.tile([128, GK, CH], BF16, tag="es", bufs=42,
                              name=f"es{qc}_{h}_{g}")
                # constant shift cancels in the softmax ratio but widens the
                # no-max-subtraction overflow envelope
                nc.scalar.activation(
                    es[:], stg[:], mybir.ActivationFunctionType.Exp,
                    bias=shift_sb[:])
                return es

            def ctx_qt(cx, h, qt, es_list):
                # one contiguous psum accumulation group per qt region:
                # the hw supports only one open accumulation group at a time
                for kt in range(NKT):
                    g, j = kt // GK, kt % GK
                    nc.tensor.matmul(
                        cx[:, qt, :],
                        lhsT=es_list[g][:, j, KT * qt:KT * (qt + 1)],
                        rhs=v_sb[kt][:, h, :],
                        start=(kt == 0), stop=(kt == NKT - 1))

            ctxn_t = {}

            def normalize(qc, h, cx):
                pt, par = h // 2, h % 2
                rec = wrk.tile([128, 4, 1], F32, tag="rec", bufs=2,
                               name=f"rec{qc}_{h}")
                nc.vector.reciprocal(rec[:], cx[:, :, HD:VW])
                if par == 0:
                    ctxn_t[(qc, pt)] = [
                        wrk.tile([128, 2, HD], BF16, tag="ctxn", bufs=6,
                                 name=f"ctxn{qc}_{pt}_{qt}")
                        for qt in range(4)]
                for qt in range(4):
                    nc.vector.tensor_scalar_mul(
                        ctxn_t[(qc, pt)][qt][:, par, :],
                        cx[:, qt, 0:HD], rec[:, qt, :])
                if par == 1:
                    # transpose [q, ch] -> [ch, q] via a single-shot matmul
                    # against the bf16 identity (ctxn^T @ I); the DMA xbar
                    # transpose's output is invisible to the dependency
                    # scheduler, so it cannot be used here
                    for qt in range(4):
                        pst = psp.tile([128, CH], F32, tag="aux", bufs=2,
                                       name=f"pst{qc}_{pt}_{qt}")
                        nc.tensor.matmul(
                            pst[:, 0:128], lhsT=ctxn_t[(qc, pt)][qt][:],
                            rhs=eyebf_sb[:], start=True, stop=True)
                        nc.vector.tensor_copy(
                            ctxT[pt][:, qc * CH + KT * qt:qc * CH + KT * (qt + 1)],
                            pst[:, 0:128])

            def oproj_piece(qc, nt, oc, tail=False):
                n0 = qc * CH + nt * KT
                if tail:
                    # the stg tag is dead in the tail: use its banks so the
                    # final pieces pipeline instead of serializing on aux
                    stile = psp.tile([128, GK, CH], F32, tag="stg", bufs=2,
                                     name=f"psot{qc}_{nt}_{oc}")
                    pso = stile[:, oc, :]
                else:
                    pso = psp.tile([128, CH], F32, tag="aux", bufs=2,
                                   name=f"pso{qc}_{nt}_{oc}")
                for it in range(2):
                    nc.tensor.matmul(
                        pso[:], lhsT=ctxT[it][:, n0:n0 + KT],
                        rhs=wo_sb[:, it, CH * oc:CH * (oc + 1)],
                        start=(it == 0), stop=(it == 1))
                ob = wrk.tile([128, CH], BF16, tag="ob", bufs=3,
                              name=f"ob{qc}_{nt}_{oc}")
                # in the tail ACT is idle: alternate the psum->sbuf copies
                # across DVE and ACT so pieces drain twice as fast
                if tail and (nt + oc) % 2 == 1:
                    nc.scalar.copy(ob[:], pso[:])
                else:
                    nc.vector.tensor_copy(ob[:], pso[:])
                nc.sync.dma_start(
                    out[n0:n0 + KT, CH * oc:CH * (oc + 1)], ob[:])

            # ======== Phase A: projections + scores/exp for qc0 ========
            # scores for group g need krot chunk g (256 tokens = one group)
            # and qrot chunks 0..1 (q tokens 0:512), so chunk c >= 1 emits
            # group c (and chunk 1 additionally emits group 0)
            xts = {0: xt0, 1: xt1}
            for c in range(NPCH):
                if c >= 2:
                    xts[c] = load_chunk_x(c)
                xt = xts[c]
                cs = cs0 if c == 0 else (cs1 if c == 1 else load_cs(c))
                proj_rope(wk_sb, 1, krot, xt, cs, c)
                proj_rope(wq_sb, 0, qrot, xt, cs, c)
                gs = [] if c == 0 else ([0, 1] if c == 1 else [c])
                for h in PILE_HEADS:
                    for g in gs:
                        es_pile[h][g] = scores_group(0, h, g)
                # V for chunk c-1 here: wv lands after x1/cs1, and shifting
                # V keeps the PE fed while the current chunk's rope chain
                # runs on the other engines
                for vc in ([c - 1] if c >= 1 else []) + ([c] if c == NPCH - 1 else []):
                    for vt in range(PCH // KT):
                        vproj(xts[vc], vc, vt)
                if c >= 1:
                    del xts[c - 1]

            nc.sync.dma_start(wo_sb[:], woT.rearrange("(t p) o -> p t o", p=128))

            # ======== Phase B: software-pipelined head chain ========
            # Each pipeline step emits the score singles for head i while the
            # ctx accumulation groups of head i-1 run between them (psum
            # accumulation groups must be contiguous; score matmuls are
            # single-shot so they may interleave). qc0 heads 0..2 (scores
            # piled in phase A) drain as whole-ctx fillers at the first
            # head boundaries; o-proj pieces drip into the g>=4 slots of
            # later heads.
            def ctx_head_filler(h):
                # full ctx+normalize for a piled qc0 head (runs between two
                # heads' pipelines: no other accumulation group is open)
                cx = psp.tile([128, 4, VW], F32, tag="cx", bufs=2,
                              name=f"cx0_{h}")
                for qt in range(4):
                    ctx_qt(cx, h, qt, es_pile[h])
                normalize(0, h, cx)

            chain = [(qc, h) for qc in range(1, NCH) for h in range(HPC)]
            boundary = {0: 0, 1: 1, 2: 2, 3: 3}  # chain idx -> piled head
            # o-proj drip: chain idx -> list of (qc, nt) pieces; qc0 ctxT is
            # complete after ci=3, qc1 after ci=4, qc2 after ci=8
            drip = {4: [(0, 0), (0, 1)], 5: [(0, 2), (0, 3)],
                    6: [(1, 0), (1, 1)], 7: [(1, 2), (1, 3)],
                    9: [(2, 0), (2, 1)], 10: [(2, 2), (2, 3)]}
            prev = None
            for ci, (qc, h) in enumerate(chain):
                es = [None] * NGR
                pieces = []
                for p_qc, p_nt in drip.get(ci, []):
                    pieces += [(p_qc, p_nt, 0), (p_qc, p_nt, 1)]
                for g in range(NGR):
                    es[g] = scores_group(qc, h, g)
                    if prev is not None and g < 4:
                        ctx_qt(prev[2], prev[1], g, prev[3])
                    elif pieces and g >= 4:
                        pc = pieces.pop(0)
                        oproj_piece(*pc)
                        if pieces:
                            oproj_piece(*pieces.pop(0))
                if prev is not None:
                    normalize(prev[0], prev[1], prev[2])
                if ci in boundary:
                    ctx_head_filler(boundary[ci])
                cx = psp.tile([128, 4, VW], F32, tag="cx", bufs=2,
                              name=f"cx{qc}_{h}")
                prev = (qc, h, cx, es)

            # tail: last head (qc3, h3, par=1) fused per q-tile so ctx,
            # normalize, transpose and o-proj pipeline instead of
            # serializing head-at-once
            t_qc, t_h, t_cx, t_es = prev
            t_pt = t_h // 2
            for qt in range(4):
                ctx_qt(t_cx, t_h, qt, t_es)
                rec = wrk.tile([128, 1, 1], F32, tag="rec", bufs=2,
                               name=f"rect{qt}")
                nc.vector.reciprocal(rec[:], t_cx[:, qt:qt + 1, HD:VW])
                ctn = ctxn_t[(t_qc, t_pt)][qt]
                nc.vector.tensor_scalar_mul(
                    ctn[:, 1, :], t_cx[:, qt, 0:HD], rec[:, 0, :])
                pst = psp.tile([128, CH], F32, tag="aux", bufs=2,
                               name=f"pstt{qt}")
                nc.tensor.matmul(
                    pst[:, 0:128], lhsT=ctn[:], rhs=eyebf_sb[:],
                    start=True, stop=True)
                nc.scalar.copy(
                    ctxT[t_pt][:, t_qc * CH + KT * qt:t_qc * CH + KT * (qt + 1)],
                    pst[:, 0:128])
                oproj_piece(NCH - 1, qt, 0, tail=True)
                oproj_piece(NCH - 1, qt, 1, tail=True)

    nc.compile()
    return nc


def _get_nc():
    if "nc" not in _CACHE:
        _CACHE["nc"] = _build()
    return _CACHE["nc"]


def _host_prep(x, rope_cos, rope_sin, Wq, bq, Wk, bk, Wv, bv, Wo, bo):
    import ml_dtypes
    perm64 = np.concatenate([np.arange(0, 64, 2), np.arange(1, 64, 2)])
    f = np.float32
    bf = ml_dtypes.bfloat16
    in_maps = []
    eyesw = np.zeros((128, 128), f)
    for c in range(128):
        eyesw[c, c ^ 32] = 1.0
    eyebf = np.eye(128, dtype=f).astype(bf)
    sign = np.tile(np.repeat(np.array([-1.0, 1.0], f), 32), C // 64)
    for core in range(DP * TP):
        b, r = divmod(core, TP)
        sel = np.concatenate([64 * (HPC * r + s) + perm64 for s in range(HPC)])
        xT = np.ascontiguousarray(x[b].T)
        cosT = np.ascontiguousarray(rope_cos[b][:, sel].T)
        sinT = np.ascontiguousarray(rope_sin[b][:, sel].T) * sign[:, None]
        wq_ = np.ascontiguousarray(Wq[sel, :].T)
        wk_ = np.ascontiguousarray(Wk[sel, :].T)
        wvx = np.zeros((D, HPC * VW), f)
        for s in range(HPC):
            cols = sel[64 * s:64 * (s + 1)]
            wvx[:, VW * s:VW * s + HD] = Wv[cols, :].T
        bqk = np.stack([bq[sel].reshape(2, 128), bk[sel].reshape(2, 128)])
        woT = np.ascontiguousarray(Wo[:, sel].T).astype(bf)
        in_maps.append({
            "xT": xT, "cosT": cosT, "sinT": sinT.astype(f),
            "wq": wq_, "wk": wk_, "wvx": wvx,
            "bqk": bqk.astype(f), "woT": woT, "eyesw": eyesw, "eyebf": eyebf,
        })
    return in_maps


def kernel(x, rope_cos, rope_sin, Wq, bq, Wk, bk, Wv, bv, Wo, bo):
    nc = _get_nc()
    in_maps = _host_prep(np.asarray(x), np.asarray(rope_cos),
                         np.asarray(rope_sin), np.asarray(Wq), np.asarray(bq),
                         np.asarray(Wk), np.asarray(bk), np.asarray(Wv),
                         np.asarray(bv), np.asarray(Wo), np.asarray(bo))
    res = bass_utils.run_bass_kernel_spmd(
        nc, in_maps, core_ids=list(range(DP * TP)))
    out = np.zeros((B, N, D), np.float32)
    for core in range(DP * TP):
        b = core // TP
        out[b] += res.results[core]["out"]
    # V bias folded into the output bias: probs sum to 1 after normalize
    bias = np.asarray(bo) + np.asarray(bv) @ np.asarray(Wo).T
    out += bias[None, None, :]
    return out


# revision 45
# speedup vs baseline: 1.0056x; 1.0056x over previous
"""Multi-head attention (B=2, N=2048, D=1024, 16 heads x 64) on 8 NeuronCores.

Sharding: data-parallel over batch (2) x tensor-parallel over heads (4 heads
per core). Each core computes q/k/v projections + RoPE + attention for its 4
heads and a partial output projection; the host sums the 4 tensor-parallel
partials per batch and adds the output bias (V-projection bias folded in).

Kernel structure (cost-model-driven):
 - Projections in f32r (full-rate at free>=256); RoPE rotate-pair via a
   channel-permuted eye matmul (permutation folded into weights host-side).
 - Scores computed transposed S^T[k, q] from bf16 q/k; exp on ACT with a
   constant -20 bias (cancels in the softmax ratio); es output in bf16.
 - ctx matmuls flipped to out[q, d] orientation (out partitions = q tokens,
   free = 65 = 64 v-cols + ones column for the denominator) in bf16: bf16
   streams 1 row/cycle at any free size, halving ctx PE time vs the
   [65, q]-oriented f32r version.
 - Softmax denominators land per-partition -> normalize is a cheap DVE
   reciprocal + tensor_scalar multiply; normalized ctx transposed for the
   output projection by single-shot matmuls against a bf16 identity (the
   DMA xbar transpose's output is invisible to the dependency scheduler).
 - Output projection in bf16; bf16 partials DMA'd out, host sums in f32.
 - The ACT exp wall (~133us) is the global bottleneck: scores+exp for all
   of q-chunk 0 are emitted during the projection phase (es tiles piled in
   SBUF, 256-token projection chunks free the SBUF for the pile), and wave
   B runs a software-pipelined head chain: head i's single-shot score
   matmuls interleave with head i-1's contiguous ctx accumulation groups
   (hw allows only one open psum accumulation group at a time), with
   o-proj pieces dripped into the remaining slots. The final head fuses
   ctx/normalize/transpose/o-proj per q-tile to shorten the tail.
"""
import sys

sys.path.insert(0, "/opt/trn_rl_repo")

import numpy as np

import concourse.bacc as bacc
import concourse.mybir as mybir
import concourse.tile as tile
from concourse import bass_utils

B, N, D = 2, 2048, 1024
HEADS, HD = 16, 64
TP = 4                 # tensor-parallel ways (heads)
DP = 2                 # data-parallel ways (batch)
HPC = HEADS // TP      # heads per core = 4
C = HPC * HD           # channels per core = 256
CH = 512               # q-chunk size (attention)
NCH = N // CH          # 4
PCH = 256              # projection x-chunk size (phase A)
NPCH = N // PCH        # 8
KT = 128               # k tile
NKT = N // KT          # 16
GK = 2                 # k-tiles per exp group
NGR = NKT // GK        # 8
VW = HD + 1            # V columns per head incl. ones column = 65
ITC = D // KT          # 8 contraction tiles for projections
F32R = mybir.dt.float32r
F32 = mybir.dt.float32
BF16 = mybir.dt.bfloat16

_CACHE = {}


def _build():
    nc = bacc.Bacc("TRN2", debug=False, num_devices=DP * TP)

    xT = nc.dram_tensor("xT", [D, N], F32R, kind="ExternalInput").ap()
    cosT = nc.dram_tensor("cosT", [C, N], F32R, kind="ExternalInput").ap()
    sinT = nc.dram_tensor("sinT", [C, N], F32R, kind="ExternalInput").ap()
    wq = nc.dram_tensor("wq", [D, C], F32R, kind="ExternalInput").ap()
    wk = nc.dram_tensor("wk", [D, C], F32R, kind="ExternalInput").ap()
    wvx = nc.dram_tensor("wvx", [D, HPC * VW], F32R, kind="ExternalInput").ap()
    bqk = nc.dram_tensor("bqk", [2, 2, 128], F32, kind="ExternalInput").ap()
    woT = nc.dram_tensor("woT", [C, D], BF16, kind="ExternalInput").ap()
    eyesw = nc.dram_tensor("eyesw", [128, 128], F32R, kind="ExternalInput").ap()
    eyebf = nc.dram_tensor("eyebf", [128, 128], BF16, kind="ExternalInput").ap()
    out = nc.dram_tensor("out", [N, D], BF16, kind="ExternalOutput").ap()

    with tile.TileContext(nc) as tc:
        with tc.tile_pool(name="pers", bufs=1) as pers, \
             tc.tile_pool(name="wrk", bufs=1) as wrk, \
             tc.tile_pool(name="psp", bufs=1, space="PSUM") as psp:
            # ---- persistent SBUF; DMA order = arrival priority: the rope
            # chain of chunk 0 gates the first exp, so wk/x0/wq/bqk/eye/cs0
            # land first and everything else queues behind ----
            bqk_sb = pers.tile([128, 2, 2], F32, tag="bqk")
            nc.sync.dma_start(bqk_sb[:], bqk.rearrange("a c p -> p a c"))
            eye_sb = pers.tile([128, 128], F32R, tag="eyesw")
            nc.sync.dma_start(eye_sb[:], eyesw)
            wk_sb = pers.tile([128, ITC, C], F32R, tag="wk")
            nc.sync.dma_start(
                wk_sb[:, 0:4, :],
                wk[0:512, :].rearrange("(t p) c -> p t c", p=128))
            nc.sync.dma_start(
                wk_sb[:, 4:, :],
                wk[512:, :].rearrange("(t p) c -> p t c", p=128))
            xt0 = wrk.tile([128, ITC, PCH], F32R, tag="xt", bufs=3, name="xt0")
            nc.sync.dma_start(
                xt0[:, 0:4, :],
                xT[0:512, 0:PCH].rearrange("(t p) n -> p t n", p=128))
            nc.sync.dma_start(
                xt0[:, 4:, :],
                xT[512:, 0:PCH].rearrange("(t p) n -> p t n", p=128))

            def load_cs(nch):
                cs = []
                ns = slice(nch * PCH, (nch + 1) * PCH)
                for t in range(2):
                    co = wrk.tile([128, PCH], F32R, tag=f"cos{t}", bufs=2,
                                  name=f"cos{t}_{nch}")
                    nc.sync.dma_start(co[:], cosT[128 * t:128 * (t + 1), ns])
                    si = wrk.tile([128, PCH], F32R, tag=f"sin{t}", bufs=2,
                                  name=f"sin{t}_{nch}")
                    nc.sync.dma_start(si[:], sinT[128 * t:128 * (t + 1), ns])
                    cs.append((co, si))
                return cs

            def load_chunk_x(nch):
                xt = wrk.tile([128, ITC, PCH], F32R, tag="xt", bufs=3)
                ns = slice(nch * PCH, (nch + 1) * PCH)
                nc.sync.dma_start(
                    xt[:, 0:4, :],
                    xT[0:512, ns].rearrange("(t p) n -> p t n", p=128))
                nc.sync.dma_start(
                    xt[:, 4:, :],
                    xT[512:, ns].rearrange("(t p) n -> p t n", p=128))
                return xt

            cs0 = load_cs(0)
            wq_sb = pers.tile([128, ITC, C], F32R, tag="wq")
            nc.sync.dma_start(
                wq_sb[:, 0:4, :],
                wq[0:512, :].rearrange("(t p) c -> p t c", p=128))
            nc.sync.dma_start(
                wq_sb[:, 4:, :],
                wq[512:, :].rearrange("(t p) c -> p t c", p=128))
            xt1 = load_chunk_x(1)
            cs1 = load_cs(1)

            wv_sb = pers.tile([128, ITC, HPC * VW], F32R, tag="wv")
            nc.sync.dma_start(wv_sb[:], wvx.rearrange("(t p) c -> p t c", p=128))
            eyebf_sb = pers.tile([128, 128], BF16, tag="eyebf")
            nc.sync.dma_start(eyebf_sb[:], eyebf)

            shift_sb = pers.tile([128, 1], F32, tag="shift")
            nc.gpsimd.memset(shift_sb[:], -20.0)
            # v tiles: persistent, bf16, ones column memset once
            v_sb = [pers.tile([128, HPC, VW], BF16, tag=f"v{t}", name=f"v{t}")
                    for t in range(NKT)]
            for t in range(NKT):
                nc.gpsimd.memset(v_sb[t][:, :, HD:VW], 1.0)

            # p-state warmup: keep the PE streak alive through the initial
            # DMA wait so the first projections run at full clock; operands
            # track the DMA arrival order (wk first, then the x halves)
            wps = psp.tile([128, CH], F32, tag="aux", bufs=2, name="warmup")
            for wi in range(10):
                nc.tensor.matmul(wps[:, 0:128], lhsT=wk_sb[:, 0, 0:128],
                                 rhs=wk_sb[:, 0, 0:128],
                                 start=True, stop=True)
            for wi in range(10):
                nc.tensor.matmul(wps[:, 0:128], lhsT=wk_sb[:, 0, 0:128],
                                 rhs=xt0[:, 0, 0:128],
                                 start=True, stop=True)
            for wi in range(6):
                nc.tensor.matmul(wps[:, 0:128], lhsT=wk_sb[:, 0, 0:128],
                                 rhs=xt0[:, ITC - 1, 0:128],
                                 start=True, stop=True)

            qrot = [pers.tile([128, N], F32R, tag=f"qrot{t}", name=f"qrot{t}")
                    for t in range(2)]
            krot = [pers.tile([128, N], F32R, tag=f"krot{t}", name=f"krot{t}")
                    for t in range(2)]
            ctxT = [pers.tile([128, N], BF16, tag=f"ctxT{t}", name=f"ctxT{t}")
                    for t in range(2)]
            wo_sb = pers.tile([128, 2, D], BF16, tag="wo")

            def proj_rope(w_sb, qk, dst, xt, cs, nch):
                ns = slice(nch * PCH, (nch + 1) * PCH)
                ps, raw = [], []
                for dc in range(2):
                    p = psp.tile([128, CH], F32, tag="aux", bufs=2,
                                 name=f"ps{qk}_{nch}_{dc}")
                    for it in range(ITC):
                        nc.tensor.matmul(
                            p[:, 0:PCH],
                            lhsT=w_sb[:, it, 128 * dc:128 * (dc + 1)],
                            rhs=xt[:, it, :],
                            start=(it == 0), stop=(it == ITC - 1))
                    ps.append(p)
                    r = wrk.tile([128, PCH], F32R, tag="raw", bufs=3,
                                 name=f"raw{qk}_{nch}_{dc}")
                    nc.vector.tensor_scalar_add(
                        r[:], p[:, 0:PCH], bqk_sb[:, qk, dc:dc + 1])
                    raw.append(r)
                for dc in range(2):
                    pssh = psp.tile([128, CH], F32, tag="aux", bufs=2,
                                    name=f"pssh{qk}_{nch}_{dc}")
                    nc.tensor.matmul(pssh[:, 0:PCH], lhsT=eye_sb[:],
                                     rhs=raw[dc][:],
                                     start=True, stop=True)
                    co, si = cs[dc]
                    # m1 and the final add are SBUF-only -> gpsimd, except
                    # for the first two chunks where the rope tail gates the
                    # first scores/exp: run those on the faster DVE. m2
                    # reads PSUM so it always stays on DVE.
                    meng = nc.vector if nch < 2 else nc.gpsimd
                    m1 = wrk.tile([128, PCH], F32, tag="m1", bufs=1)
                    meng.tensor_mul(m1[:], raw[dc][:], co[:])
                    m2 = wrk.tile([128, PCH], F32, tag="m2", bufs=1)
                    nc.vector.tensor_mul(m2[:], pssh[:, 0:PCH], si[:])
                    meng.tensor_add(dst[dc][:, ns], m1[:], m2[:])

            def vproj(xt, nch, vt):
                kt = nch * (PCH // KT) + vt
                psv = psp.tile([128, CH], F32, tag="aux", bufs=2,
                               name=f"psv{kt}")
                for it in range(ITC):
                    nc.tensor.matmul(
                        psv[:, 0:HPC * VW],
                        lhsT=xt[:, it, KT * vt:KT * (vt + 1)],
                        rhs=wv_sb[:, it, :],
                        start=(it == 0), stop=(it == ITC - 1))
                pv = psv[:, 0:HPC * VW].rearrange("p (s w) -> p s w", w=VW)
                nc.vector.tensor_copy(v_sb[kt][:, :, 0:HD], pv[:, :, 0:HD])

            # scores/exp for qc0 heads 0,1,3 run during phase A; their es
            # tiles pile up in SBUF until wave B consumes them. Head 2 leads
            # the wave-B pipeline so that within each head-pair the par=0
            # head normalizes before the par=1 head (the par=1 normalize
            # completes the pair's ctxn tiles and fires the transposes).
            PILE_HEADS = (0, 1, 2, 3)
            es_pile = {h: [None] * NGR for h in PILE_HEADS}

            def scores_group(qc, h, g):
                pt, par = h // 2, h % 2
                r0 = 64 * par
                qs = slice(qc * CH, (qc + 1) * CH)
                stg = psp.tile([128, GK, CH], F32, tag="stg", bufs=2,
                               name=f"stg{qc}_{h}_{g}")
                for j in range(GK):
                    kt = GK * g + j
                    nc.tensor.matmul(
                        stg[:, j, :],
                        lhsT=krot[pt][r0:r0 + 64, KT * kt:KT * (kt + 1)],
                        rhs=qrot[pt][r0:r0 + 64, qs],
                        start=True, stop=True)
                es = wrk.tile([128, GK, CH], BF16, tag="es", bufs=42,
                              name=f"es{qc}_{h}_{g}")
                # constant shift cancels in the softmax ratio but widens the
                # no-max-subtraction overflow envelope
                nc.scalar.activation(
                    es[:], stg[:], mybir.ActivationFunctionType.Exp,
                    bias=shift_sb[:])
                return es

            def ctx_qt(cx, h, qt, es_list):
                # one contiguous psum accumulation group per qt region:
                # the hw supports only one open accumulation group at a time
                for kt in range(NKT):
                    g, j = kt // GK, kt % GK
                    nc.tensor.matmul(
                        cx[:, qt, :],
                        lhsT=es_list[g][:, j, KT * qt:KT * (qt + 1)],
                        rhs=v_sb[kt][:, h, :],
                        start=(kt == 0), stop=(kt == NKT - 1))

            ctxn_t = {}

            def normalize(qc, h, cx):
                pt, par = h // 2, h % 2
                rec = wrk.tile([128, 4, 1], F32, tag="rec", bufs=2,
                               name=f"rec{qc}_{h}")
                nc.vector.reciprocal(rec[:], cx[:, :, HD:VW])
                if par == 0:
                    ctxn_t[(qc, pt)] = [
                        wrk.tile([128, 2, HD], BF16, tag="ctxn", bufs=6,
                                 name=f"ctxn{qc}_{pt}_{qt}")
                        for qt in range(4)]
                for qt in range(4):
                    nc.vector.tensor_scalar_mul(
                        ctxn_t[(qc, pt)][qt][:, par, :],
                        cx[:, qt, 0:HD], rec[:, qt, :])
                if par == 1:
                    # transpose [q, ch] -> [ch, q] via a single-shot matmul
                    # against the bf16 identity (ctxn^T @ I); the DMA xbar
                    # transpose's output is invisible to the dependency
                    # scheduler, so it cannot be used here
                    for qt in range(4):
                        pst = psp.tile([128, CH], F32, tag="aux", bufs=2,
                                       name=f"pst{qc}_{pt}_{qt}")
                        nc.tensor.matmul(
                            pst[:, 0:128], lhsT=ctxn_t[(qc, pt)][qt][:],
                            rhs=eyebf_sb[:], start=True, stop=True)
                        nc.vector.tensor_copy(
                            ctxT[pt][:, qc * CH + KT * qt:qc * CH + KT * (qt + 1)],
                            pst[:, 0:128])

            def oproj_piece(qc, nt, oc, tail=False):
                n0 = qc * CH + nt * KT
                if tail:
                    # the stg tag is dead in the tail: use its banks so the
                    # final pieces pipeline instead of serializing on aux
                    stile = psp.tile([128, GK, CH], F32, tag="stg", bufs=2,
                                     name=f"psot{qc}_{nt}_{oc}")
                    pso = stile[:, oc, :]
                else:
                    pso = psp.tile([128, CH], F32, tag="aux", bufs=2,
                                   name=f"pso{qc}_{nt}_{oc}")
                for it in range(2):
                    nc.tensor.matmul(
                        pso[:], lhsT=ctxT[it][:, n0:n0 + KT],
                        rhs=wo_sb[:, it, CH * oc:CH * (oc + 1)],
                        start=(it == 0), stop=(it == 1))
                ob = wrk.tile([128, CH], BF16, tag="ob", bufs=3,
                              name=f"ob{qc}_{nt}_{oc}")
                # in the tail ACT is idle: alternate the psum->sbuf copies
                # across DVE and ACT so pieces drain twice as fast
                if tail and (nt + oc) % 2 == 1:
                    nc.scalar.copy(ob[:], pso[:])
                else:
                    nc.vector.tensor_copy(ob[:], pso[:])
                nc.sync.dma_start(
                    out[n0:n0 + KT, CH * oc:CH * (oc + 1)], ob[:])

            # ======== Phase A: projections + scores/exp for qc0 ========
            # scores for group g need krot chunk g (256 tokens = one group)
            # and qrot chunks 0..1 (q tokens 0:512), so chunk c >= 1 emits
            # group c (and chunk 1 additionally emits group 0)
            xts = {0: xt0, 1: xt1}
            for c in range(NPCH):
                if c >= 2:
                    xts[c] = load_chunk_x(c)
                xt = xts[c]
                cs = cs0 if c == 0 else (cs1 if c == 1 else load_cs(c))
                proj_rope(wk_sb, 1, krot, xt, cs, c)
                proj_rope(wq_sb, 0, qrot, xt, cs, c)
                gs = [] if c == 0 else ([0, 1] if c == 1 else [c])
                for h in PILE_HEADS:
                    for g in gs:
                        es_pile[h][g] = scores_group(0, h, g)
                # V for chunk c-1 here: wv lands after x1/cs1, and shifting
                # V keeps the PE fed while the current chunk's rope chain
                # runs on the other engines
                for vc in ([c - 1] if c >= 1 else []) + ([c] if c == NPCH - 1 else []):
                    for vt in range(PCH // KT):
                        vproj(xts[vc], vc, vt)
                if c >= 1:
                    del xts[c - 1]

            nc.sync.dma_start(wo_sb[:], woT.rearrange("(t p) o -> p t o", p=128))

            # ======== Phase B: software-pipelined head chain ========
            # Each pipeline step emits the score singles for head i while the
            # ctx accumulation groups of head i-1 run between them (psum
            # accumulation groups must be contiguous; score matmuls are
            # single-shot so they may interleave). qc0 heads 0..2 (scores
            # piled in phase A) drain as whole-ctx fillers at the first
            # head boundaries; o-proj pieces drip into the g>=4 slots of
            # later heads.
            def ctx_head_filler(h):
                # full ctx+normalize for a piled qc0 head (runs between two
                # heads' pipelines: no other accumulation group is open)
                cx = psp.tile([128, 4, VW], F32, tag="cx", bufs=2,
                              name=f"cx0_{h}")
                for qt in range(4):
                    ctx_qt(cx, h, qt, es_pile[h])
                normalize(0, h, cx)

            chain = [(qc, h) for qc in range(1, NCH) for h in range(HPC)]
            boundary = {0: 0, 1: 1, 2: 2, 3: 3}  # chain idx -> piled head
            # o-proj drip: chain idx -> list of (qc, nt) pieces; qc0 ctxT is
            # complete after ci=3, qc1 after ci=4, qc2 after ci=8
            drip = {4: [(0, 0), (0, 1)], 5: [(0, 2), (0, 3)],
                    6: [(1, 0), (1, 1)], 7: [(1, 2), (1, 3)],
                    9: [(2, 0), (2, 1)], 10: [(2, 2), (2, 3)]}
            prev = None
            for ci, (qc, h) in enumerate(chain):
                es = [None] * NGR
                pieces = []
                for p_qc, p_nt in drip.get(ci, []):
                    pieces += [(p_qc, p_nt, 0), (p_qc, p_nt, 1)]
                for g in range(NGR):
                    es[g] = scores_group(qc, h, g)
                    if prev is not None and g < 4:
                        ctx_qt(prev[2], prev[1], g, prev[3])
                    elif pieces and g >= 4:
                        pc = pieces.pop(0)
                        oproj_piece(*pc)
                        if pieces:
                            oproj_piece(*pieces.pop(0))
                if prev is not None:
                    normalize(prev[0], prev[1], prev[2])
                if ci in boundary:
                    ctx_head_filler(boundary[ci])
                cx = psp.tile([128, 4, VW], F32, tag="cx", bufs=2,
                              name=f"cx{qc}_{h}")
                prev = (qc, h, cx, es)

            # tail: last head (qc3, h3, par=1) fused per q-tile so ctx,
            # normalize, transpose and o-proj pipeline instead of
            # serializing head-at-once
            t_qc, t_h, t_cx, t_es = prev
            t_pt = t_h // 2
            for qt in range(4):
                ctx_qt(t_cx, t_h, qt, t_es)
                rec = wrk.tile([128, 1, 1], F32, tag="rec", bufs=2,
                               name=f"rect{qt}")
                nc.vector.reciprocal(rec[:], t_cx[:, qt:qt + 1, HD:VW])
                ctn = ctxn_t[(t_qc, t_pt)][qt]
                nc.vector.tensor_scalar_mul(
                    ctn[:, 1, :], t_cx[:, qt, 0:HD], rec[:, 0, :])
                pst = psp.tile([128, CH], F32, tag="aux", bufs=2,
                               name=f"pstt{qt}")
                nc.tensor.matmul(
                    pst[:, 0:128], lhsT=ctn[:], rhs=eyebf_sb[:],
                    start=True, stop=True)
                nc.scalar.copy(
                    ctxT[t_pt][:, t_qc * CH + KT * qt:t_qc * CH + KT * (qt + 1)],
                    pst[:, 0:128])
                oproj_piece(NCH - 1, qt, 0, tail=True)
                oproj_piece(NCH - 1, qt, 1, tail=True)

    nc.compile()
    return nc


def _get_nc():
    if "nc" not in _CACHE:
        _CACHE["nc"] = _build()
    return _CACHE["nc"]


def _host_prep(x, rope_cos, rope_sin, Wq, bq, Wk, bk, Wv, bv, Wo, bo):
    import ml_dtypes
    perm64 = np.concatenate([np.arange(0, 64, 2), np.arange(1, 64, 2)])
    f = np.float32
    bf = ml_dtypes.bfloat16
    in_maps = []
    eyesw = np.zeros((128, 128), f)
    for c in range(128):
        eyesw[c, c ^ 32] = 1.0
    eyebf = np.eye(128, dtype=f).astype(bf)
    sign = np.tile(np.repeat(np.array([-1.0, 1.0], f), 32), C // 64)
    for core in range(DP * TP):
        b, r = divmod(core, TP)
        sel = np.concatenate([64 * (HPC * r + s) + perm64 for s in range(HPC)])
        xT = np.ascontiguousarray(x[b].T)
        cosT = np.ascontiguousarray(rope_cos[b][:, sel].T)
        sinT = np.ascontiguousarray(rope_sin[b][:, sel].T) * sign[:, None]
        wq_ = np.ascontiguousarray(Wq[sel, :].T)
        wk_ = np.ascontiguousarray(Wk[sel, :].T)
        wvx = np.zeros((D, HPC * VW), f)
        for s in range(HPC):
            cols = sel[64 * s:64 * (s + 1)]
            wvx[:, VW * s:VW * s + HD] = Wv[cols, :].T
        bqk = np.stack([bq[sel].reshape(2, 128), bk[sel].reshape(2, 128)])
        woT = np.ascontiguousarray(Wo[:, sel].T).astype(bf)
        in_maps.append({
            "xT": xT, "cosT": cosT, "sinT": sinT.astype(f),
            "wq": wq_, "wk": wk_, "wvx": wvx,
            "bqk": bqk.astype(f), "woT": woT, "eyesw": eyesw, "eyebf": eyebf,
        })
    return in_maps


def kernel(x, rope_cos, rope_sin, Wq, bq, Wk, bk, Wv, bv, Wo, bo):
    nc = _get_nc()
    in_maps = _host_prep(np.asarray(x), np.asarray(rope_cos),
                         np.asarray(rope_sin), np.asarray(Wq), np.asarray(bq),
                         np.asarray(Wk), np.asarray(bk), np.asarray(Wv),
                         np.asarray(bv), np.asarray(Wo), np.asarray(bo))
    res = bass_utils.run_bass_kernel_spmd(
        nc, in_maps, core_ids=list(range(DP * TP)))
    out = np.zeros((B, N, D), np.float32)
    for core in range(DP * TP):
        b = core // TP
        out[b] += res.results[core]["out"]
    # V bias folded into the output bias: probs sum to 1 after normalize
    bias = np.asarray(bo) + np.asarray(bv) @ np.asarray(Wo).T
    out += bias[None, None, :]
    return out


# revision 46
# speedup vs baseline: 1.0077x; 1.0022x over previous
"""Multi-head attention (B=2, N=2048, D=1024, 16 heads x 64) on 8 NeuronCores.

Sharding: data-parallel over batch (2) x tensor-parallel over heads (4 heads
per core). Each core computes q/k/v projections + RoPE + attention for its 4
heads and a partial output projection; the host sums the 4 tensor-parallel
partials per batch and adds the output bias (V-projection bias folded in).

Kernel structure (cost-model-driven):
 - Projections in f32r (full-rate at free>=256); RoPE rotate-pair via a
   channel-permuted eye matmul (permutation folded into weights host-side).
 - Scores computed transposed S^T[k, q] from bf16 q/k; exp on ACT with a
   constant -20 bias (cancels in the softmax ratio); es output in bf16.
 - ctx matmuls flipped to out[q, d] orientation (out partitions = q tokens,
   free = 65 = 64 v-cols + ones column for the denominator) in bf16: bf16
   streams 1 row/cycle at any free size, halving ctx PE time vs the
   [65, q]-oriented f32r version.
 - Softmax denominators land per-partition -> normalize is a cheap DVE
   reciprocal + tensor_scalar multiply; normalized ctx transposed for the
   output projection by single-shot matmuls against a bf16 identity (the
   DMA xbar transpose's output is invisible to the dependency scheduler).
 - Output projection in bf16; bf16 partials DMA'd out, host sums in f32.
 - The ACT exp wall (~133us) is the global bottleneck: scores+exp for all
   of q-chunk 0 are emitted during the projection phase (es tiles piled in
   SBUF, 256-token projection chunks free the SBUF for the pile), and wave
   B runs a software-pipelined head chain: head i's single-shot score
   matmuls interleave with head i-1's contiguous ctx accumulation groups
   (hw allows only one open psum accumulation group at a time), with
   o-proj pieces dripped into the remaining slots. The final head fuses
   ctx/normalize/transpose/o-proj per q-tile to shorten the tail.
"""
import sys

sys.path.insert(0, "/opt/trn_rl_repo")

import numpy as np

import concourse.bacc as bacc
import concourse.mybir as mybir
import concourse.tile as tile
from concourse import bass_utils

B, N, D = 2, 2048, 1024
HEADS, HD = 16, 64
TP = 4                 # tensor-parallel ways (heads)
DP = 2                 # data-parallel ways (batch)
HPC = HEADS // TP      # heads per core = 4
C = HPC * HD           # channels per core = 256
CH = 512               # q-chunk size (attention)
NCH = N // CH          # 4
PCH = 256              # projection x-chunk size (phase A)
NPCH = N // PCH        # 8
KT = 128               # k tile
NKT = N // KT          # 16
GK = 2                 # k-tiles per exp group
NGR = NKT // GK        # 8
VW = HD + 1            # V columns per head incl. ones column = 65
ITC = D // KT          # 8 contraction tiles for projections
F32R = mybir.dt.float32r
F32 = mybir.dt.float32
BF16 = mybir.dt.bfloat16

_CACHE = {}


def _build():
    nc = bacc.Bacc("TRN2", debug=False, num_devices=DP * TP)

    xT = nc.dram_tensor("xT", [D, N], F32R, kind="ExternalInput").ap()
    cosT = nc.dram_tensor("cosT", [C, N], F32R, kind="ExternalInput").ap()
    sinT = nc.dram_tensor("sinT", [C, N], F32R, kind="ExternalInput").ap()
    wq = nc.dram_tensor("wq", [D, C], F32R, kind="ExternalInput").ap()
    wk = nc.dram_tensor("wk", [D, C], F32R, kind="ExternalInput").ap()
    wvx = nc.dram_tensor("wvx", [D, HPC * VW], F32R, kind="ExternalInput").ap()
    bqk = nc.dram_tensor("bqk", [2, 2, 128], F32, kind="ExternalInput").ap()
    woT = nc.dram_tensor("woT", [C, D], BF16, kind="ExternalInput").ap()
    eyesw = nc.dram_tensor("eyesw", [128, 128], F32R, kind="ExternalInput").ap()
    eyebf = nc.dram_tensor("eyebf", [128, 128], BF16, kind="ExternalInput").ap()
    out = nc.dram_tensor("out", [N, D], BF16, kind="ExternalOutput").ap()

    with tile.TileContext(nc) as tc:
        with tc.tile_pool(name="pers", bufs=1) as pers, \
             tc.tile_pool(name="wrk", bufs=1) as wrk, \
             tc.tile_pool(name="psp", bufs=1, space="PSUM") as psp:
            # ---- persistent SBUF; DMA order = arrival priority: the rope
            # chain of chunk 0 gates the first exp, so wk/x0/wq/bqk/eye/cs0
            # land first and everything else queues behind ----
            bqk_sb = pers.tile([128, 2, 2], F32, tag="bqk")
            nc.sync.dma_start(bqk_sb[:], bqk.rearrange("a c p -> p a c"))
            eye_sb = pers.tile([128, 128], F32R, tag="eyesw")
            nc.sync.dma_start(eye_sb[:], eyesw)
            wk_sb = pers.tile([128, ITC, C], F32R, tag="wk")
            nc.sync.dma_start(
                wk_sb[:, 0:4, :],
                wk[0:512, :].rearrange("(t p) c -> p t c", p=128))
            nc.sync.dma_start(
                wk_sb[:, 4:, :],
                wk[512:, :].rearrange("(t p) c -> p t c", p=128))
            xt0 = wrk.tile([128, ITC, PCH], F32R, tag="xt", bufs=3, name="xt0")
            nc.sync.dma_start(
                xt0[:, 0:4, :],
                xT[0:512, 0:PCH].rearrange("(t p) n -> p t n", p=128))
            nc.sync.dma_start(
                xt0[:, 4:, :],
                xT[512:, 0:PCH].rearrange("(t p) n -> p t n", p=128))

            def load_cs(nch):
                cs = []
                ns = slice(nch * PCH, (nch + 1) * PCH)
                for t in range(2):
                    co = wrk.tile([128, PCH], F32R, tag=f"cos{t}", bufs=2,
                                  name=f"cos{t}_{nch}")
                    nc.sync.dma_start(co[:], cosT[128 * t:128 * (t + 1), ns])
                    si = wrk.tile([128, PCH], F32R, tag=f"sin{t}", bufs=2,
                                  name=f"sin{t}_{nch}")
                    nc.sync.dma_start(si[:], sinT[128 * t:128 * (t + 1), ns])
                    cs.append((co, si))
                return cs

            def load_chunk_x(nch):
                xt = wrk.tile([128, ITC, PCH], F32R, tag="xt", bufs=3)
                ns = slice(nch * PCH, (nch + 1) * PCH)
                nc.sync.dma_start(
                    xt[:, 0:4, :],
                    xT[0:512, ns].rearrange("(t p) n -> p t n", p=128))
                nc.sync.dma_start(
                    xt[:, 4:, :],
                    xT[512:, ns].rearrange("(t p) n -> p t n", p=128))
                return xt

            cs0 = load_cs(0)
            wq_sb = pers.tile([128, ITC, C], F32R, tag="wq")
            nc.sync.dma_start(
                wq_sb[:, 0:4, :],
                wq[0:512, :].rearrange("(t p) c -> p t c", p=128))
            nc.sync.dma_start(
                wq_sb[:, 4:, :],
                wq[512:, :].rearrange("(t p) c -> p t c", p=128))
            xt1 = load_chunk_x(1)
            cs1 = load_cs(1)

            wv_sb = pers.tile([128, ITC, HPC * VW], F32R, tag="wv")
            nc.sync.dma_start(wv_sb[:], wvx.rearrange("(t p) c -> p t c", p=128))
            eyebf_sb = pers.tile([128, 128], BF16, tag="eyebf")
            nc.sync.dma_start(eyebf_sb[:], eyebf)

            shift_sb = pers.tile([128, 1], F32, tag="shift")
            nc.gpsimd.memset(shift_sb[:], -20.0)
            # v tiles: persistent, bf16, ones column memset once
            v_sb = [pers.tile([128, HPC, VW], BF16, tag=f"v{t}", name=f"v{t}")
                    for t in range(NKT)]
            for t in range(NKT):
                nc.gpsimd.memset(v_sb[t][:, :, HD:VW], 1.0)

            # p-state warmup: keep the PE streak alive through the initial
            # DMA wait so the first projections run at full clock; operands
            # track the DMA arrival order (wk first, then the x halves)
            wps = psp.tile([128, CH], F32, tag="aux", bufs=2, name="warmup")
            for wi in range(10):
                nc.tensor.matmul(wps[:, 0:128], lhsT=wk_sb[:, 0, 0:128],
                                 rhs=wk_sb[:, 0, 0:128],
                                 start=True, stop=True)
            for wi in range(10):
                nc.tensor.matmul(wps[:, 0:128], lhsT=wk_sb[:, 0, 0:128],
                                 rhs=xt0[:, 0, 0:128],
                                 start=True, stop=True)
            for wi in range(6):
                nc.tensor.matmul(wps[:, 0:128], lhsT=wk_sb[:, 0, 0:128],
                                 rhs=xt0[:, ITC - 1, 0:128],
                                 start=True, stop=True)

            qrot = [pers.tile([128, N], F32R, tag=f"qrot{t}", name=f"qrot{t}")
                    for t in range(2)]
            krot = [pers.tile([128, N], F32R, tag=f"krot{t}", name=f"krot{t}")
                    for t in range(2)]
            ctxT = [pers.tile([128, N], BF16, tag=f"ctxT{t}", name=f"ctxT{t}")
                    for t in range(2)]
            wo_sb = pers.tile([128, 2, D], BF16, tag="wo")

            def proj_rope(w_sb, qk, dst, xt, cs, nch):
                ns = slice(nch * PCH, (nch + 1) * PCH)
                ps, raw = [], []
                for dc in range(2):
                    p = psp.tile([128, CH], F32, tag="aux", bufs=2,
                                 name=f"ps{qk}_{nch}_{dc}")
                    for it in range(ITC):
                        nc.tensor.matmul(
                            p[:, 0:PCH],
                            lhsT=w_sb[:, it, 128 * dc:128 * (dc + 1)],
                            rhs=xt[:, it, :],
                            start=(it == 0), stop=(it == ITC - 1))
                    ps.append(p)
                    r = wrk.tile([128, PCH], F32R, tag="raw", bufs=3,
                                 name=f"raw{qk}_{nch}_{dc}")
                    # chunks 0-1 gate the first exp; their serial rope chain
                    # spreads across three engines (bias->ACT, muls->DVE,
                    # add->Pool) instead of queueing 32 ops on DVE alone
                    if nch < 2:
                        nc.scalar.activation(
                            r[:], p[:, 0:PCH],
                            mybir.ActivationFunctionType.Identity,
                            bias=bqk_sb[:, qk, dc:dc + 1])
                    else:
                        nc.vector.tensor_scalar_add(
                            r[:], p[:, 0:PCH], bqk_sb[:, qk, dc:dc + 1])
                    raw.append(r)
                for dc in range(2):
                    pssh = psp.tile([128, CH], F32, tag="aux", bufs=2,
                                    name=f"pssh{qk}_{nch}_{dc}")
                    nc.tensor.matmul(pssh[:, 0:PCH], lhsT=eye_sb[:],
                                     rhs=raw[dc][:],
                                     start=True, stop=True)
                    co, si = cs[dc]
                    # m1 and the final add are SBUF-only -> gpsimd, except
                    # for the first two chunks where the rope tail gates the
                    # first scores/exp: run those on the faster DVE. m2
                    # reads PSUM so it always stays on DVE.
                    m1 = wrk.tile([128, PCH], F32, tag="m1", bufs=1)
                    aeng = nc.gpsimd
                    if nch < 2:
                        nc.vector.tensor_mul(m1[:], raw[dc][:], co[:])
                    else:
                        nc.gpsimd.tensor_mul(m1[:], raw[dc][:], co[:])
                    m2 = wrk.tile([128, PCH], F32, tag="m2", bufs=1)
                    nc.vector.tensor_mul(m2[:], pssh[:, 0:PCH], si[:])
                    aeng.tensor_add(dst[dc][:, ns], m1[:], m2[:])

            def vproj(xt, nch, vt):
                kt = nch * (PCH // KT) + vt
                psv = psp.tile([128, CH], F32, tag="aux", bufs=2,
                               name=f"psv{kt}")
                for it in range(ITC):
                    nc.tensor.matmul(
                        psv[:, 0:HPC * VW],
                        lhsT=xt[:, it, KT * vt:KT * (vt + 1)],
                        rhs=wv_sb[:, it, :],
                        start=(it == 0), stop=(it == ITC - 1))
                pv = psv[:, 0:HPC * VW].rearrange("p (s w) -> p s w", w=VW)
                nc.vector.tensor_copy(v_sb[kt][:, :, 0:HD], pv[:, :, 0:HD])

            # scores/exp for qc0 heads 0,1,3 run during phase A; their es
            # tiles pile up in SBUF until wave B consumes them. Head 2 leads
            # the wave-B pipeline so that within each head-pair the par=0
            # head normalizes before the par=1 head (the par=1 normalize
            # completes the pair's ctxn tiles and fires the transposes).
            PILE_HEADS = (0, 1, 2, 3)
            es_pile = {h: [None] * NGR for h in PILE_HEADS}

            def scores_group(qc, h, g):
                pt, par = h // 2, h % 2
                r0 = 64 * par
                qs = slice(qc * CH, (qc + 1) * CH)
                stg = psp.tile([128, GK, CH], F32, tag="stg", bufs=2,
                               name=f"stg{qc}_{h}_{g}")
                for j in range(GK):
                    kt = GK * g + j
                    nc.tensor.matmul(
                        stg[:, j, :],
                        lhsT=krot[pt][r0:r0 + 64, KT * kt:KT * (kt + 1)],
                        rhs=qrot[pt][r0:r0 + 64, qs],
                        start=True, stop=True)
                es = wrk.tile([128, GK, CH], BF16, tag="es", bufs=42,
                              name=f"es{qc}_{h}_{g}")
                # constant shift cancels in the softmax ratio but widens the
                # no-max-subtraction overflow envelope
                nc.scalar.activation(
                    es[:], stg[:], mybir.ActivationFunctionType.Exp,
                    bias=shift_sb[:])
                return es

            def ctx_qt(cx, h, qt, es_list):
                # one contiguous psum accumulation group per qt region:
                # the hw supports only one open accumulation group at a time
                for kt in range(NKT):
                    g, j = kt // GK, kt % GK
                    nc.tensor.matmul(
                        cx[:, qt, :],
                        lhsT=es_list[g][:, j, KT * qt:KT * (qt + 1)],
                        rhs=v_sb[kt][:, h, :],
                        start=(kt == 0), stop=(kt == NKT - 1))

            ctxn_t = {}

            def normalize(qc, h, cx):
                pt, par = h // 2, h % 2
                rec = wrk.tile([128, 4, 1], F32, tag="rec", bufs=2,
                               name=f"rec{qc}_{h}")
                nc.vector.reciprocal(rec[:], cx[:, :, HD:VW])
                if par == 0:
                    ctxn_t[(qc, pt)] = [
                        wrk.tile([128, 2, HD], BF16, tag="ctxn", bufs=6,
                                 name=f"ctxn{qc}_{pt}_{qt}")
                        for qt in range(4)]
                for qt in range(4):
                    nc.vector.tensor_scalar_mul(
                        ctxn_t[(qc, pt)][qt][:, par, :],
                        cx[:, qt, 0:HD], rec[:, qt, :])
                if par == 1:
                    # transpose [q, ch] -> [ch, q] via a single-shot matmul
                    # against the bf16 identity (ctxn^T @ I); the DMA xbar
                    # transpose's output is invisible to the dependency
                    # scheduler, so it cannot be used here
                    for qt in range(4):
                        pst = psp.tile([128, CH], F32, tag="aux", bufs=2,
                                       name=f"pst{qc}_{pt}_{qt}")
                        nc.tensor.matmul(
                            pst[:, 0:128], lhsT=ctxn_t[(qc, pt)][qt][:],
                            rhs=eyebf_sb[:], start=True, stop=True)
                        nc.vector.tensor_copy(
                            ctxT[pt][:, qc * CH + KT * qt:qc * CH + KT * (qt + 1)],
                            pst[:, 0:128])

            def oproj_piece(qc, nt, oc, tail=False):
                n0 = qc * CH + nt * KT
                if tail:
                    # the stg tag is dead in the tail: use its banks so the
                    # final pieces pipeline instead of serializing on aux
                    stile = psp.tile([128, GK, CH], F32, tag="stg", bufs=2,
                                     name=f"psot{qc}_{nt}_{oc}")
                    pso = stile[:, oc, :]
                else:
                    pso = psp.tile([128, CH], F32, tag="aux", bufs=2,
                                   name=f"pso{qc}_{nt}_{oc}")
                for it in range(2):
                    nc.tensor.matmul(
                        pso[:], lhsT=ctxT[it][:, n0:n0 + KT],
                        rhs=wo_sb[:, it, CH * oc:CH * (oc + 1)],
                        start=(it == 0), stop=(it == 1))
                ob = wrk.tile([128, CH], BF16, tag="ob", bufs=3,
                              name=f"ob{qc}_{nt}_{oc}")
                # in the tail ACT is idle: alternate the psum->sbuf copies
                # across DVE and ACT so pieces drain twice as fast
                if tail and (nt + oc) % 2 == 1:
                    nc.scalar.copy(ob[:], pso[:])
                else:
                    nc.vector.tensor_copy(ob[:], pso[:])
                nc.sync.dma_start(
                    out[n0:n0 + KT, CH * oc:CH * (oc + 1)], ob[:])

            # ======== Phase A: projections + scores/exp for qc0 ========
            # scores for group g need krot chunk g (256 tokens = one group)
            # and qrot chunks 0..1 (q tokens 0:512), so chunk c >= 1 emits
            # group c (and chunk 1 additionally emits group 0)
            xts = {0: xt0, 1: xt1}
            for c in range(NPCH):
                if c >= 2:
                    xts[c] = load_chunk_x(c)
                xt = xts[c]
                cs = cs0 if c == 0 else (cs1 if c == 1 else load_cs(c))
                proj_rope(wk_sb, 1, krot, xt, cs, c)
                proj_rope(wq_sb, 0, qrot, xt, cs, c)
                gs = [] if c == 0 else ([0, 1] if c == 1 else [c])
                for h in PILE_HEADS:
                    for g in gs:
                        es_pile[h][g] = scores_group(0, h, g)
                # V for chunk c-1 here: wv lands after x1/cs1, and shifting
                # V keeps the PE fed while the current chunk's rope chain
                # runs on the other engines
                for vc in ([c - 1] if c >= 1 else []) + ([c] if c == NPCH - 1 else []):
                    for vt in range(PCH // KT):
                        vproj(xts[vc], vc, vt)
                if c >= 1:
                    del xts[c - 1]

            nc.sync.dma_start(wo_sb[:], woT.rearrange("(t p) o -> p t o", p=128))

            # ======== Phase B: software-pipelined head chain ========
            # Each pipeline step emits the score singles for head i while the
            # ctx accumulation groups of head i-1 run between them (psum
            # accumulation groups must be contiguous; score matmuls are
            # single-shot so they may interleave). qc0 heads 0..2 (scores
            # piled in phase A) drain as whole-ctx fillers at the first
            # head boundaries; o-proj pieces drip into the g>=4 slots of
            # later heads.
            def ctx_head_filler(h):
                # full ctx+normalize for a piled qc0 head (runs between two
                # heads' pipelines: no other accumulation group is open)
                cx = psp.tile([128, 4, VW], F32, tag="cx", bufs=2,
                              name=f"cx0_{h}")
                for qt in range(4):
                    ctx_qt(cx, h, qt, es_pile[h])
                normalize(0, h, cx)

            chain = [(qc, h) for qc in range(1, NCH) for h in range(HPC)]
            boundary = {0: 0, 1: 1, 2: 2, 3: 3}  # chain idx -> piled head
            # o-proj drip: chain idx -> list of (qc, nt) pieces; qc0 ctxT is
            # complete after ci=3, qc1 after ci=4, qc2 after ci=8
            drip = {4: [(0, 0), (0, 1)], 5: [(0, 2), (0, 3)],
                    6: [(1, 0), (1, 1)], 7: [(1, 2), (1, 3)],
                    9: [(2, 0), (2, 1)], 10: [(2, 2), (2, 3)]}
            prev = None
            for ci, (qc, h) in enumerate(chain):
                es = [None] * NGR
                pieces = []
                for p_qc, p_nt in drip.get(ci, []):
                    pieces += [(p_qc, p_nt, 0), (p_qc, p_nt, 1)]
                for g in range(NGR):
                    es[g] = scores_group(qc, h, g)
                    if prev is not None and g < 4:
                        ctx_qt(prev[2], prev[1], g, prev[3])
                    elif pieces and g >= 4:
                        pc = pieces.pop(0)
                        oproj_piece(*pc)
                        if pieces:
                            oproj_piece(*pieces.pop(0))
                if prev is not None:
                    normalize(prev[0], prev[1], prev[2])
                if ci in boundary:
                    ctx_head_filler(boundary[ci])
                cx = psp.tile([128, 4, VW], F32, tag="cx", bufs=2,
                              name=f"cx{qc}_{h}")
                prev = (qc, h, cx, es)

            # tail: last head (qc3, h3, par=1) fused per q-tile so ctx,
            # normalize, transpose and o-proj pipeline instead of
            # serializing head-at-once
            t_qc, t_h, t_cx, t_es = prev
            t_pt = t_h // 2
            for qt in range(4):
                ctx_qt(t_cx, t_h, qt, t_es)
                rec = wrk.tile([128, 1, 1], F32, tag="rec", bufs=2,
                               name=f"rect{qt}")
                nc.vector.reciprocal(rec[:], t_cx[:, qt:qt + 1, HD:VW])
                ctn = ctxn_t[(t_qc, t_pt)][qt]
                nc.vector.tensor_scalar_mul(
                    ctn[:, 1, :], t_cx[:, qt, 0:HD], rec[:, 0, :])
                pst = psp.tile([128, CH], F32, tag="aux", bufs=2,
                               name=f"pstt{qt}")
                nc.tensor.matmul(
                    pst[:, 0:128], lhsT=ctn[:], rhs=eyebf_sb[:],
                    start=True, stop=True)
                nc.scalar.copy(
                    ctxT[t_pt][:, t_qc * CH + KT * qt:t_qc * CH + KT * (qt + 1)],
                    pst[:, 0:128])
                oproj_piece(NCH - 1, qt, 0, tail=True)
                oproj_piece(NCH - 1, qt, 1, tail=True)

    nc.compile()
    return nc


def _get_nc():
    if "nc" not in _CACHE:
        _CACHE["nc"] = _build()
    return _CACHE["nc"]


def _host_prep(x, rope_cos, rope_sin, Wq, bq, Wk, bk, Wv, bv, Wo, bo):
    import ml_dtypes
    perm64 = np.concatenate([np.arange(0, 64, 2), np.arange(1, 64, 2)])
    f = np.float32
    bf = ml_dtypes.bfloat16
    in_maps = []
    eyesw = np.zeros((128, 128), f)
    for c in range(128):
        eyesw[c, c ^ 32] = 1.0
    eyebf = np.eye(128, dtype=f).astype(bf)
    sign = np.tile(np.repeat(np.array([-1.0, 1.0], f), 32), C // 64)
    for core in range(DP * TP):
        b, r = divmod(core, TP)
        sel = np.concatenate([64 * (HPC * r + s) + perm64 for s in range(HPC)])
        xT = np.ascontiguousarray(x[b].T)
        cosT = np.ascontiguousarray(rope_cos[b][:, sel].T)
        sinT = np.ascontiguousarray(rope_sin[b][:, sel].T) * sign[:, None]
        wq_ = np.ascontiguousarray(Wq[sel, :].T)
        wk_ = np.ascontiguousarray(Wk[sel, :].T)
        wvx = np.zeros((D, HPC * VW), f)
        for s in range(HPC):
            cols = sel[64 * s:64 * (s + 1)]
            wvx[:, VW * s:VW * s + HD] = Wv[cols, :].T
        bqk = np.stack([bq[sel].reshape(2, 128), bk[sel].reshape(2, 128)])
        woT = np.ascontiguousarray(Wo[:, sel].T).astype(bf)
        in_maps.append({
            "xT": xT, "cosT": cosT, "sinT": sinT.astype(f),
            "wq": wq_, "wk": wk_, "wvx": wvx,
            "bqk": bqk.astype(f), "woT": woT, "eyesw": eyesw, "eyebf": eyebf,
        })
    return in_maps


def kernel(x, rope_cos, rope_sin, Wq, bq, Wk, bk, Wv, bv, Wo, bo):
    nc = _get_nc()
    in_maps = _host_prep(np.asarray(x), np.asarray(rope_cos),
                         np.asarray(rope_sin), np.asarray(Wq), np.asarray(bq),
                         np.asarray(Wk), np.asarray(bk), np.asarray(Wv),
                         np.asarray(bv), np.asarray(Wo), np.asarray(bo))
    res = bass_utils.run_bass_kernel_spmd(
        nc, in_maps, core_ids=list(range(DP * TP)))
    out = np.zeros((B, N, D), np.float32)
    for core in range(DP * TP):
        b = core // TP
        out[b] += res.results[core]["out"]
    # V bias folded into the output bias: probs sum to 1 after normalize
    bias = np.asarray(bo) + np.asarray(bv) @ np.asarray(Wo).T
    out += bias[None, None, :]
    return out


# revision 47
# speedup vs baseline: 1.0082x; 1.0004x over previous
"""Multi-head attention (B=2, N=2048, D=1024, 16 heads x 64) on 8 NeuronCores.

Sharding: data-parallel over batch (2) x tensor-parallel over heads (4 heads
per core). Each core computes q/k/v projections + RoPE + attention for its 4
heads and a partial output projection; the host sums the 4 tensor-parallel
partials per batch and adds the output bias (V-projection bias folded in).

Kernel structure (cost-model-driven):
 - Projections in f32r (full-rate at free>=256); RoPE rotate-pair via a
   channel-permuted eye matmul (permutation folded into weights host-side).
 - Scores computed transposed S^T[k, q] from bf16 q/k; exp on ACT with a
   constant -20 bias (cancels in the softmax ratio); es output in bf16.
 - ctx matmuls flipped to out[q, d] orientation (out partitions = q tokens,
   free = 65 = 64 v-cols + ones column for the denominator) in bf16: bf16
   streams 1 row/cycle at any free size, halving ctx PE time vs the
   [65, q]-oriented f32r version.
 - Softmax denominators land per-partition -> normalize is a cheap DVE
   reciprocal + tensor_scalar multiply; normalized ctx transposed for the
   output projection by single-shot matmuls against a bf16 identity (the
   DMA xbar transpose's output is invisible to the dependency scheduler).
 - Output projection in bf16; bf16 partials DMA'd out, host sums in f32.
 - The ACT exp wall (~133us) is the global bottleneck: scores+exp for all
   of q-chunk 0 are emitted during the projection phase (es tiles piled in
   SBUF, 256-token projection chunks free the SBUF for the pile), and wave
   B runs a software-pipelined head chain: head i's single-shot score
   matmuls interleave with head i-1's contiguous ctx accumulation groups
   (hw allows only one open psum accumulation group at a time), with
   o-proj pieces dripped into the remaining slots. The final head fuses
   ctx/normalize/transpose/o-proj per q-tile to shorten the tail.
"""
import sys

sys.path.insert(0, "/opt/trn_rl_repo")

import numpy as np

import concourse.bacc as bacc
import concourse.mybir as mybir
import concourse.tile as tile
from concourse import bass_utils

B, N, D = 2, 2048, 1024
HEADS, HD = 16, 64
TP = 4                 # tensor-parallel ways (heads)
DP = 2                 # data-parallel ways (batch)
HPC = HEADS // TP      # heads per core = 4
C = HPC * HD           # channels per core = 256
CH = 512               # q-chunk size (attention)
NCH = N // CH          # 4
PCH = 256              # projection x-chunk size (phase A)
NPCH = N // PCH        # 8
KT = 128               # k tile
NKT = N // KT          # 16
GK = 2                 # k-tiles per exp group
NGR = NKT // GK        # 8
VW = HD + 1            # V columns per head incl. ones column = 65
ITC = D // KT          # 8 contraction tiles for projections
F32R = mybir.dt.float32r
F32 = mybir.dt.float32
BF16 = mybir.dt.bfloat16

_CACHE = {}


def _build():
    nc = bacc.Bacc("TRN2", debug=False, num_devices=DP * TP)

    xT = nc.dram_tensor("xT", [D, N], F32R, kind="ExternalInput").ap()
    cosT = nc.dram_tensor("cosT", [C, N], F32R, kind="ExternalInput").ap()
    sinT = nc.dram_tensor("sinT", [C, N], F32R, kind="ExternalInput").ap()
    wq = nc.dram_tensor("wq", [D, C], F32R, kind="ExternalInput").ap()
    wk = nc.dram_tensor("wk", [D, C], F32R, kind="ExternalInput").ap()
    wvx = nc.dram_tensor("wvx", [D, HPC * VW], F32R, kind="ExternalInput").ap()
    bqk = nc.dram_tensor("bqk", [2, 2, 128], F32, kind="ExternalInput").ap()
    woT = nc.dram_tensor("woT", [C, D], BF16, kind="ExternalInput").ap()
    eyesw = nc.dram_tensor("eyesw", [128, 128], F32R, kind="ExternalInput").ap()
    eyebf = nc.dram_tensor("eyebf", [128, 128], BF16, kind="ExternalInput").ap()
    out = nc.dram_tensor("out", [N, D], BF16, kind="ExternalOutput").ap()

    with tile.TileContext(nc) as tc:
        with tc.tile_pool(name="pers", bufs=1) as pers, \
             tc.tile_pool(name="wrk", bufs=1) as wrk, \
             tc.tile_pool(name="psp", bufs=1, space="PSUM") as psp:
            # ---- persistent SBUF; DMA order = arrival priority: the rope
            # chain of chunk 0 gates the first exp, so wk/x0/wq/bqk/eye/cs0
            # land first and everything else queues behind ----
            bqk_sb = pers.tile([128, 2, 2], F32, tag="bqk")
            nc.sync.dma_start(bqk_sb[:], bqk.rearrange("a c p -> p a c"))
            eye_sb = pers.tile([128, 128], F32R, tag="eyesw")
            nc.sync.dma_start(eye_sb[:], eyesw)
            wk_sb = pers.tile([128, ITC, C], F32R, tag="wk")
            nc.sync.dma_start(
                wk_sb[:, 0:4, :],
                wk[0:512, :].rearrange("(t p) c -> p t c", p=128))
            nc.sync.dma_start(
                wk_sb[:, 4:, :],
                wk[512:, :].rearrange("(t p) c -> p t c", p=128))
            xt0 = wrk.tile([128, ITC, PCH], F32R, tag="xt", bufs=3, name="xt0")
            nc.sync.dma_start(
                xt0[:, 0:4, :],
                xT[0:512, 0:PCH].rearrange("(t p) n -> p t n", p=128))
            nc.sync.dma_start(
                xt0[:, 4:, :],
                xT[512:, 0:PCH].rearrange("(t p) n -> p t n", p=128))

            def load_cs(nch):
                cs = []
                ns = slice(nch * PCH, (nch + 1) * PCH)
                for t in range(2):
                    co = wrk.tile([128, PCH], F32R, tag=f"cos{t}", bufs=2,
                                  name=f"cos{t}_{nch}")
                    nc.sync.dma_start(co[:], cosT[128 * t:128 * (t + 1), ns])
                    si = wrk.tile([128, PCH], F32R, tag=f"sin{t}", bufs=2,
                                  name=f"sin{t}_{nch}")
                    nc.sync.dma_start(si[:], sinT[128 * t:128 * (t + 1), ns])
                    cs.append((co, si))
                return cs

            def load_chunk_x(nch):
                xt = wrk.tile([128, ITC, PCH], F32R, tag="xt", bufs=3)
                ns = slice(nch * PCH, (nch + 1) * PCH)
                nc.sync.dma_start(
                    xt[:, 0:4, :],
                    xT[0:512, ns].rearrange("(t p) n -> p t n", p=128))
                nc.sync.dma_start(
                    xt[:, 4:, :],
                    xT[512:, ns].rearrange("(t p) n -> p t n", p=128))
                return xt

            cs0 = load_cs(0)
            wq_sb = pers.tile([128, ITC, C], F32R, tag="wq")
            nc.sync.dma_start(
                wq_sb[:, 0:4, :],
                wq[0:512, :].rearrange("(t p) c -> p t c", p=128))
            nc.sync.dma_start(
                wq_sb[:, 4:, :],
                wq[512:, :].rearrange("(t p) c -> p t c", p=128))
            xt1 = load_chunk_x(1)
            cs1 = load_cs(1)

            wv_sb = pers.tile([128, ITC, HPC * VW], F32R, tag="wv")
            nc.sync.dma_start(wv_sb[:], wvx.rearrange("(t p) c -> p t c", p=128))
            eyebf_sb = pers.tile([128, 128], BF16, tag="eyebf")
            nc.sync.dma_start(eyebf_sb[:], eyebf)

            shift_sb = pers.tile([128, 1], F32, tag="shift")
            nc.gpsimd.memset(shift_sb[:], -20.0)
            # v tiles: persistent, bf16, ones column memset once
            v_sb = [pers.tile([128, HPC, VW], BF16, tag=f"v{t}", name=f"v{t}")
                    for t in range(NKT)]
            for t in range(NKT):
                nc.gpsimd.memset(v_sb[t][:, :, HD:VW], 1.0)

            # p-state warmup: keep the PE streak alive through the initial
            # DMA wait so the first projections run at full clock; operands
            # track the DMA arrival order (wk first, then the x halves)
            wps = psp.tile([128, CH], F32, tag="aux", bufs=2, name="warmup")
            for wi in range(10):
                nc.tensor.matmul(wps[:, 0:128], lhsT=wk_sb[:, 0, 0:128],
                                 rhs=wk_sb[:, 0, 0:128],
                                 start=True, stop=True)
            for wi in range(10):
                nc.tensor.matmul(wps[:, 0:128], lhsT=wk_sb[:, 0, 0:128],
                                 rhs=xt0[:, 0, 0:128],
                                 start=True, stop=True)
            for wi in range(6):
                nc.tensor.matmul(wps[:, 0:128], lhsT=wk_sb[:, 0, 0:128],
                                 rhs=xt0[:, ITC - 1, 0:128],
                                 start=True, stop=True)

            qrot = [pers.tile([128, N], F32R, tag=f"qrot{t}", name=f"qrot{t}")
                    for t in range(2)]
            krot = [pers.tile([128, N], F32R, tag=f"krot{t}", name=f"krot{t}")
                    for t in range(2)]
            ctxT = [pers.tile([128, N], BF16, tag=f"ctxT{t}", name=f"ctxT{t}")
                    for t in range(2)]
            wo_sb = pers.tile([128, 2, D], BF16, tag="wo")

            def proj_rope(w_sb, qk, dst, xt, cs, nch):
                ns = slice(nch * PCH, (nch + 1) * PCH)
                ps, raw = [], []
                for dc in range(2):
                    p = psp.tile([128, CH], F32, tag="aux", bufs=2,
                                 name=f"ps{qk}_{nch}_{dc}")
                    for it in range(ITC):
                        nc.tensor.matmul(
                            p[:, 0:PCH],
                            lhsT=w_sb[:, it, 128 * dc:128 * (dc + 1)],
                            rhs=xt[:, it, :],
                            start=(it == 0), stop=(it == ITC - 1))
                    ps.append(p)
                    r = wrk.tile([128, PCH], F32R, tag="raw", bufs=3,
                                 name=f"raw{qk}_{nch}_{dc}")
                    # chunks 0-1 gate the first exp; their serial rope chain
                    # spreads across three engines (bias->ACT, muls->DVE,
                    # add->Pool) instead of queueing 32 ops on DVE alone
                    if nch < 2:
                        nc.scalar.activation(
                            r[:], p[:, 0:PCH],
                            mybir.ActivationFunctionType.Identity,
                            bias=bqk_sb[:, qk, dc:dc + 1])
                    else:
                        nc.vector.tensor_scalar_add(
                            r[:], p[:, 0:PCH], bqk_sb[:, qk, dc:dc + 1])
                    raw.append(r)
                for dc in range(2):
                    pssh = psp.tile([128, CH], F32, tag="aux", bufs=2,
                                    name=f"pssh{qk}_{nch}_{dc}")
                    nc.tensor.matmul(pssh[:, 0:PCH], lhsT=eye_sb[:],
                                     rhs=raw[dc][:],
                                     start=True, stop=True)
                    co, si = cs[dc]
                    # m1 and the final add are SBUF-only -> gpsimd, except
                    # for the first two chunks where the rope tail gates the
                    # first scores/exp: run those on the faster DVE. m2
                    # reads PSUM so it always stays on DVE.
                    m1 = wrk.tile([128, PCH], F32, tag="m1", bufs=1)
                    aeng = nc.gpsimd
                    if nch < 2:
                        nc.vector.tensor_mul(m1[:], raw[dc][:], co[:])
                    else:
                        nc.gpsimd.tensor_mul(m1[:], raw[dc][:], co[:])
                    m2 = wrk.tile([128, PCH], F32, tag="m2", bufs=1)
                    nc.vector.tensor_mul(m2[:], pssh[:, 0:PCH], si[:])
                    aeng.tensor_add(dst[dc][:, ns], m1[:], m2[:])

            def vproj(xt, nch, vt):
                kt = nch * (PCH // KT) + vt
                psv = psp.tile([128, CH], F32, tag="aux", bufs=2,
                               name=f"psv{kt}")
                for it in range(ITC):
                    nc.tensor.matmul(
                        psv[:, 0:HPC * VW],
                        lhsT=xt[:, it, KT * vt:KT * (vt + 1)],
                        rhs=wv_sb[:, it, :],
                        start=(it == 0), stop=(it == ITC - 1))
                pv = psv[:, 0:HPC * VW].rearrange("p (s w) -> p s w", w=VW)
                nc.vector.tensor_copy(v_sb[kt][:, :, 0:HD], pv[:, :, 0:HD])

            # scores/exp for qc0 heads 0,1,3 run during phase A; their es
            # tiles pile up in SBUF until wave B consumes them. Head 2 leads
            # the wave-B pipeline so that within each head-pair the par=0
            # head normalizes before the par=1 head (the par=1 normalize
            # completes the pair's ctxn tiles and fires the transposes).
            PILE_HEADS = (0, 1, 2, 3)
            es_pile = {h: [None] * NGR for h in PILE_HEADS}

            def scores_group(qc, h, g, split=False):
                pt, par = h // 2, h % 2
                r0 = 64 * par
                stg = psp.tile([128, GK, CH], F32, tag="stg", bufs=2,
                               name=f"stg{qc}_{h}_{g}")
                es = wrk.tile([128, GK, CH], BF16, tag="es", bufs=42,
                              name=f"es{qc}_{h}_{g}")
                # constant shift cancels in the softmax ratio but widens the
                # no-max-subtraction overflow envelope
                if split:
                    # q-halved: half 0 only needs x-chunk 0's rope, so the
                    # very first exp starts while chunk 1's rope still runs
                    for qh in range(2):
                        qs = slice(qc * CH + 256 * qh,
                                   qc * CH + 256 * (qh + 1))
                        for j in range(GK):
                            kt = GK * g + j
                            nc.tensor.matmul(
                                stg[:, j, 256 * qh:256 * (qh + 1)],
                                lhsT=krot[pt][r0:r0 + 64,
                                              KT * kt:KT * (kt + 1)],
                                rhs=qrot[pt][r0:r0 + 64, qs],
                                start=True, stop=True)
                        nc.scalar.activation(
                            es[:, :, 256 * qh:256 * (qh + 1)],
                            stg[:, :, 256 * qh:256 * (qh + 1)],
                            mybir.ActivationFunctionType.Exp,
                            bias=shift_sb[:])
                    return es
                qs = slice(qc * CH, (qc + 1) * CH)
                for j in range(GK):
                    kt = GK * g + j
                    nc.tensor.matmul(
                        stg[:, j, :],
                        lhsT=krot[pt][r0:r0 + 64, KT * kt:KT * (kt + 1)],
                        rhs=qrot[pt][r0:r0 + 64, qs],
                        start=True, stop=True)
                nc.scalar.activation(
                    es[:], stg[:], mybir.ActivationFunctionType.Exp,
                    bias=shift_sb[:])
                return es

            def ctx_qt(cx, h, qt, es_list):
                # one contiguous psum accumulation group per qt region:
                # the hw supports only one open accumulation group at a time
                for kt in range(NKT):
                    g, j = kt // GK, kt % GK
                    nc.tensor.matmul(
                        cx[:, qt, :],
                        lhsT=es_list[g][:, j, KT * qt:KT * (qt + 1)],
                        rhs=v_sb[kt][:, h, :],
                        start=(kt == 0), stop=(kt == NKT - 1))

            ctxn_t = {}

            def normalize(qc, h, cx):
                pt, par = h // 2, h % 2
                rec = wrk.tile([128, 4, 1], F32, tag="rec", bufs=2,
                               name=f"rec{qc}_{h}")
                nc.vector.reciprocal(rec[:], cx[:, :, HD:VW])
                if par == 0:
                    ctxn_t[(qc, pt)] = [
                        wrk.tile([128, 2, HD], BF16, tag="ctxn", bufs=6,
                                 name=f"ctxn{qc}_{pt}_{qt}")
                        for qt in range(4)]
                for qt in range(4):
                    nc.vector.tensor_scalar_mul(
                        ctxn_t[(qc, pt)][qt][:, par, :],
                        cx[:, qt, 0:HD], rec[:, qt, :])
                if par == 1:
                    # transpose [q, ch] -> [ch, q] via a single-shot matmul
                    # against the bf16 identity (ctxn^T @ I); the DMA xbar
                    # transpose's output is invisible to the dependency
                    # scheduler, so it cannot be used here
                    for qt in range(4):
                        pst = psp.tile([128, CH], F32, tag="aux", bufs=2,
                                       name=f"pst{qc}_{pt}_{qt}")
                        nc.tensor.matmul(
                            pst[:, 0:128], lhsT=ctxn_t[(qc, pt)][qt][:],
                            rhs=eyebf_sb[:], start=True, stop=True)
                        nc.vector.tensor_copy(
                            ctxT[pt][:, qc * CH + KT * qt:qc * CH + KT * (qt + 1)],
                            pst[:, 0:128])

            def oproj_piece(qc, nt, oc, tail=False):
                n0 = qc * CH + nt * KT
                if tail:
                    # the stg tag is dead in the tail: use its banks so the
                    # final pieces pipeline instead of serializing on aux
                    stile = psp.tile([128, GK, CH], F32, tag="stg", bufs=2,
                                     name=f"psot{qc}_{nt}_{oc}")
                    pso = stile[:, oc, :]
                else:
                    pso = psp.tile([128, CH], F32, tag="aux", bufs=2,
                                   name=f"pso{qc}_{nt}_{oc}")
                for it in range(2):
                    nc.tensor.matmul(
                        pso[:], lhsT=ctxT[it][:, n0:n0 + KT],
                        rhs=wo_sb[:, it, CH * oc:CH * (oc + 1)],
                        start=(it == 0), stop=(it == 1))
                ob = wrk.tile([128, CH], BF16, tag="ob", bufs=3,
                              name=f"ob{qc}_{nt}_{oc}")
                # in the tail ACT is idle: alternate the psum->sbuf copies
                # across DVE and ACT so pieces drain twice as fast
                if tail and (nt + oc) % 2 == 1:
                    nc.scalar.copy(ob[:], pso[:])
                else:
                    nc.vector.tensor_copy(ob[:], pso[:])
                nc.sync.dma_start(
                    out[n0:n0 + KT, CH * oc:CH * (oc + 1)], ob[:])

            # ======== Phase A: projections + scores/exp for qc0 ========
            # scores for group g need krot chunk g (256 tokens = one group)
            # and qrot chunks 0..1 (q tokens 0:512), so chunk c >= 1 emits
            # group c (and chunk 1 additionally emits group 0)
            xts = {0: xt0, 1: xt1}
            for c in range(NPCH):
                if c >= 2:
                    xts[c] = load_chunk_x(c)
                xt = xts[c]
                cs = cs0 if c == 0 else (cs1 if c == 1 else load_cs(c))
                proj_rope(wk_sb, 1, krot, xt, cs, c)
                proj_rope(wq_sb, 0, qrot, xt, cs, c)
                if c == 1:
                    # group 0 first across all heads, q-halved
                    for h in PILE_HEADS:
                        es_pile[h][0] = scores_group(0, h, 0, split=True)
                    for h in PILE_HEADS:
                        es_pile[h][1] = scores_group(0, h, 1)
                elif c > 1:
                    for h in PILE_HEADS:
                        es_pile[h][c] = scores_group(0, h, c)
                # V for chunk c-1 here: wv lands after x1/cs1, and shifting
                # V keeps the PE fed while the current chunk's rope chain
                # runs on the other engines
                for vc in ([c - 1] if c >= 1 else []) + ([c] if c == NPCH - 1 else []):
                    for vt in range(PCH // KT):
                        vproj(xts[vc], vc, vt)
                if c >= 1:
                    del xts[c - 1]

            nc.sync.dma_start(wo_sb[:], woT.rearrange("(t p) o -> p t o", p=128))

            # ======== Phase B: software-pipelined head chain ========
            # Each pipeline step emits the score singles for head i while the
            # ctx accumulation groups of head i-1 run between them (psum
            # accumulation groups must be contiguous; score matmuls are
            # single-shot so they may interleave). qc0 heads 0..2 (scores
            # piled in phase A) drain as whole-ctx fillers at the first
            # head boundaries; o-proj pieces drip into the g>=4 slots of
            # later heads.
            def ctx_head_filler(h):
                # full ctx+normalize for a piled qc0 head (runs between two
                # heads' pipelines: no other accumulation group is open)
                cx = psp.tile([128, 4, VW], F32, tag="cx", bufs=2,
                              name=f"cx0_{h}")
                for qt in range(4):
                    ctx_qt(cx, h, qt, es_pile[h])
                normalize(0, h, cx)

            chain = [(qc, h) for qc in range(1, NCH) for h in range(HPC)]
            boundary = {0: 0, 1: 1, 2: 2, 3: 3}  # chain idx -> piled head
            # o-proj drip: chain idx -> list of (qc, nt) pieces; qc0 ctxT is
            # complete after ci=3, qc1 after ci=4, qc2 after ci=8
            drip = {4: [(0, 0), (0, 1)], 5: [(0, 2), (0, 3)],
                    6: [(1, 0), (1, 1)], 7: [(1, 2), (1, 3)],
                    9: [(2, 0), (2, 1)], 10: [(2, 2), (2, 3)]}
            prev = None
            for ci, (qc, h) in enumerate(chain):
                es = [None] * NGR
                pieces = []
                for p_qc, p_nt in drip.get(ci, []):
                    pieces += [(p_qc, p_nt, 0), (p_qc, p_nt, 1)]
                for g in range(NGR):
                    es[g] = scores_group(qc, h, g)
                    if prev is not None and g < 4:
                        ctx_qt(prev[2], prev[1], g, prev[3])
                    elif pieces and g >= 4:
                        pc = pieces.pop(0)
                        oproj_piece(*pc)
                        if pieces:
                            oproj_piece(*pieces.pop(0))
                if prev is not None:
                    normalize(prev[0], prev[1], prev[2])
                if ci in boundary:
                    ctx_head_filler(boundary[ci])
                cx = psp.tile([128, 4, VW], F32, tag="cx", bufs=2,
                              name=f"cx{qc}_{h}")
                prev = (qc, h, cx, es)

            # tail: last head (qc3, h3, par=1) fused per q-tile so ctx,
            # normalize, transpose and o-proj pipeline instead of
            # serializing head-at-once
            t_qc, t_h, t_cx, t_es = prev
            t_pt = t_h // 2
            for qt in range(4):
                ctx_qt(t_cx, t_h, qt, t_es)
                rec = wrk.tile([128, 1, 1], F32, tag="rec", bufs=2,
                               name=f"rect{qt}")
                nc.vector.reciprocal(rec[:], t_cx[:, qt:qt + 1, HD:VW])
                ctn = ctxn_t[(t_qc, t_pt)][qt]
                nc.vector.tensor_scalar_mul(
                    ctn[:, 1, :], t_cx[:, qt, 0:HD], rec[:, 0, :])
                pst = psp.tile([128, CH], F32, tag="aux", bufs=2,
                               name=f"pstt{qt}")
                nc.tensor.matmul(
                    pst[:, 0:128], lhsT=ctn[:], rhs=eyebf_sb[:],
                    start=True, stop=True)
                nc.scalar.copy(
                    ctxT[t_pt][:, t_qc * CH + KT * qt:t_qc * CH + KT * (qt + 1)],
                    pst[:, 0:128])
                oproj_piece(NCH - 1, qt, 0, tail=True)
                oproj_piece(NCH - 1, qt, 1, tail=True)

    nc.compile()
    return nc


def _get_nc():
    if "nc" not in _CACHE:
        _CACHE["nc"] = _build()
    return _CACHE["nc"]


def _host_prep(x, rope_cos, rope_sin, Wq, bq, Wk, bk, Wv, bv, Wo, bo):
    import ml_dtypes
    perm64 = np.concatenate([np.arange(0, 64, 2), np.arange(1, 64, 2)])
    f = np.float32
    bf = ml_dtypes.bfloat16
    in_maps = []
    eyesw = np.zeros((128, 128), f)
    for c in range(128):
        eyesw[c, c ^ 32] = 1.0
    eyebf = np.eye(128, dtype=f).astype(bf)
    sign = np.tile(np.repeat(np.array([-1.0, 1.0], f), 32), C // 64)
    for core in range(DP * TP):
        b, r = divmod(core, TP)
        sel = np.concatenate([64 * (HPC * r + s) + perm64 for s in range(HPC)])
        xT = np.ascontiguousarray(x[b].T)
        cosT = np.ascontiguousarray(rope_cos[b][:, sel].T)
        sinT = np.ascontiguousarray(rope_sin[b][:, sel].T) * sign[:, None]
        wq_ = np.ascontiguousarray(Wq[sel, :].T)
        wk_ = np.ascontiguousarray(Wk[sel, :].T)
        wvx = np.zeros((D, HPC * VW), f)
        for s in range(HPC):
            cols = sel[64 * s:64 * (s + 1)]
            wvx[:, VW * s:VW * s + HD] = Wv[cols, :].T
        bqk = np.stack([bq[sel].reshape(2, 128), bk[sel].reshape(2, 128)])
        woT = np.ascontiguousarray(Wo[:, sel].T).astype(bf)
        in_maps.append({
            "xT": xT, "cosT": cosT, "sinT": sinT.astype(f),
            "wq": wq_, "wk": wk_, "wvx": wvx,
            "bqk": bqk.astype(f), "woT": woT, "eyesw": eyesw, "eyebf": eyebf,
        })
    return in_maps


def kernel(x, rope_cos, rope_sin, Wq, bq, Wk, bk, Wv, bv, Wo, bo):
    nc = _get_nc()
    in_maps = _host_prep(np.asarray(x), np.asarray(rope_cos),
                         np.asarray(rope_sin), np.asarray(Wq), np.asarray(bq),
                         np.asarray(Wk), np.asarray(bk), np.asarray(Wv),
                         np.asarray(bv), np.asarray(Wo), np.asarray(bo))
    res = bass_utils.run_bass_kernel_spmd(
        nc, in_maps, core_ids=list(range(DP * TP)))
    out = np.zeros((B, N, D), np.float32)
    for core in range(DP * TP):
        b = core // TP
        out[b] += res.results[core]["out"]
    # V bias folded into the output bias: probs sum to 1 after normalize
    bias = np.asarray(bo) + np.asarray(bv) @ np.asarray(Wo).T
    out += bias[None, None, :]
    return out
